# revision 25
# baseline (speedup 1.0000x reference)
"""Trainium2 Bass kernel for nn_DiffKS (differentiable Karplus-Strong).

Structure of the computation:
  y[t] = x[t] - sum_{j=0..5} vals[t,j] * y[t - 1 - z_l[t] - j]
with vals / z_l derived from spline-interpolated delay & coefficient
trajectories.  The feedback lag (1 + z_l + j) is always >= ~93 samples, so
128-sample chunks can be computed as dense banded matmuls against a
512-sample window of past output plus a small within-chunk correction.

Parallel structure (v2):
  - 32 time segments of 2048 samples; each of the 8 cores runs 4 segments
    as INDEPENDENT interleaved chunk-chains (4 chains x 16 rounds), which
    keeps the tensor engine busy while each chain's serial
    matmul->ring-update link completes.  All tensors fp16 (weights, ring,
    H) with fp32 PSUM accumulation: host-simulated rel err ~5e-4.
  - phase B (parallel): each chain runs its segment's chunked recurrence
    with basis+1 right-hand sides (unit initial-window columns + one
    particular column).  Ring columns double as the response operator H
    (streamed to DRAM in fp16); the corrected final windows form the
    segment's transfer operator T.
  - combine (host, tiny): chain the 32 transfer operators to get every
    segment's true initial window.
  - apply (parallel): y[:, c] = H_c @ [w; 1] as fused multiply+
    accumulate-reduce ops split across the Vector and GpSimd engines,
    plus within-chunk correction fix-ups.

Weights are pre-negated on the host so the serial ring update is a plain
PSUM->SBUF copy on the Scalar (ACT) engine (plus a [128,1] x-column add on
Vector), keeping the Vector engine off the critical chain.
"""

import os
import numpy as np

import concourse.bacc as bacc
import concourse.tile as tile
import concourse.mybir as mybir
from concourse.bass_utils import run_bass_kernel_spmd


def _ensure_ntff_hook():
    """The agent image's `antenv` stub lacks `axon_hooks`, which
    `run_bass_kernel_spmd(trace=True)` needs under axon for NTFF capture."""
    try:
        from antenv.axon_hooks import get_axon_ntff_profile_hook  # noqa: F401
        return
    except ImportError:
        pass
    import contextlib
    import ctypes
    import sys
    import types

    so_path = "/opt/axon/libaxon_pjrt.so"
    if not os.path.exists(so_path):
        return
    lib = ctypes.CDLL(so_path)
    if not hasattr(lib, "axon_start_nrt_profile"):
        return
    lib.axon_start_nrt_profile.argtypes = [
        ctypes.POINTER(ctypes.c_int64), ctypes.c_size_t]
    lib.axon_start_nrt_profile.restype = ctypes.c_int64
    lib.axon_stop_nrt_profile.argtypes = [ctypes.c_char_p]
    lib.axon_stop_nrt_profile.restype = ctypes.c_int64

    @contextlib.contextmanager
    def _hook(output_dir, device_ids):
        import jax
        jax.devices()
        if device_ids:
            ids = (ctypes.c_int64 * len(device_ids))(*device_ids)
            rc = lib.axon_start_nrt_profile(ids, len(device_ids))
        else:
            rc = lib.axon_start_nrt_profile(None, 0)
        if rc != 0:
            raise RuntimeError(f"axon_start_nrt_profile rc={rc}")
        try:
            yield
        finally:
            n = lib.axon_stop_nrt_profile(str(output_dir).encode())
            if n <= 0:
                print(f"ntff profile: {n} file(s) written to {output_dir}",
                      file=sys.stderr)

    mod = types.ModuleType("antenv.axon_hooks")
    mod._hook = _hook
    mod.get_axon_ntff_profile_hook = lambda: _hook
    mod.set_axon_ntff_profile_hook = lambda h: setattr(mod, "_hook", h)
    import antenv
    antenv.axon_hooks = mod
    sys.modules["antenv.axon_hooks"] = mod


_ensure_ntff_hook()

F32 = mybir.dt.float32
F16 = mybir.dt.float16

N_SAMPLES = 65536
N_FRAMES = 64
L_ORDER = 5
CHUNK = 128
WIN = 512            # window length the chunk matmuls see (4 ring cols)
RING = 8             # ring columns per chain in SBUF
CORR = 64            # within-chunk correction width (needs z_l >= 63)
N_CORES = 8
CH = 4               # independent chains (segments) per core
CPC = 16             # chunks (rounds) per chain
SEGS = N_CORES * CH  # 32 segments of 2048 samples

# filled by kernel() with per-phase profiling results for the test harness
LAST_RESULTS = {}

_NC_CACHE = {}

# device rhs layout: basis columns [0, basis); one zero pad column; the
# particular column at PIDX (4-byte aligned for the fp16 column update);
# one trailing pad so the total width is even.
def _nr_of(basis):
    pidx = basis + 1 + (basis + 1) % 2
    return pidx + 2 - (basis + 1) % 2, pidx


def _seg_of(s, q):
    """Segment index handled by core s, chain q."""
    return s + N_CORES * q


# ----------------------------------------------------------------------------
# host-side preprocessing
# ----------------------------------------------------------------------------

_SPLINE_CACHE = {}


def _spline_matrix(n_in, n_out):
    """Static [n_out, n_in] natural-cubic-spline interpolation matrix."""
    key = (n_in, n_out)
    if key in _SPLINE_CACHE:
        return _SPLINE_CACHE[key]
    t_in = np.linspace(0.0, 1.0, n_in)
    t_out = np.linspace(0.0, 1.0, n_out)
    n = n_in
    h = t_in[1:] - t_in[:-1]
    R = np.zeros((n - 2, n))
    for i in range(n - 2):
        R[i, i] += 6.0 / h[i]
        R[i, i + 1] += -6.0 / h[i] - 6.0 / h[i + 1]
        R[i, i + 2] += 6.0 / h[i + 1]
    A = (
        np.diag(2.0 * (h[:-1] + h[1:]))
        + np.diag(h[1:-1], 1)
        + np.diag(h[1:-1], -1)
    )
    M = np.zeros((n, n))
    M[1:-1] = np.linalg.solve(A, R)
    idx = np.clip(np.searchsorted(t_in, t_out, side="right") - 1, 0, n - 2)
    dt = t_out - t_in[idx]
    S = np.zeros((n_out, n))
    eye = np.eye(n)
    for r in range(n_out):
        i = idx[r]
        b = (eye[i + 1] - eye[i]) / h[i] - h[i] * (2.0 * M[i] + M[i + 1]) / 6.0
        c = M[i] / 2.0
        d = (M[i + 1] - M[i]) / (6.0 * h[i])
        S[r] = eye[i] + b * dt[r] + c * dt[r] ** 2 + d * dt[r] ** 3
    S = S.astype(np.float32)
    _SPLINE_CACHE[key] = S
    return S


def _preprocess(delay, raw, exc, n_samples):
    sig = 1.0 / (1.0 + np.exp(-np.asarray(raw, np.float32)))
    coeff = sig / sig.sum(-1, keepdims=True)
    S = _spline_matrix(N_FRAMES, n_samples)
    delay_interp = S @ np.asarray(delay, np.float32)
    coeff_interp = S @ coeff
    z_l = np.floor(delay_interp).astype(np.int32)
    alfa = (delay_interp - z_l).astype(np.float32)
    b = coeff_interp
    v0 = -(1.0 - alfa) * b[:, 0]
    vmid = -(alfa[:, None] * b[:, : L_ORDER - 1]
             + (1.0 - alfa)[:, None] * b[:, 1:L_ORDER])
    vL = -alfa * b[:, -1]
    vals = np.concatenate([v0[:, None], vmid, vL[:, None]], 1).astype(np.float32)
    x = np.zeros(n_samples, np.float32)
    exc = np.asarray(exc, np.float32)
    x[: exc.shape[0]] = exc
    return vals, z_l, x


def _build_wts(vals, z_l, n_samples):
    """Dense per-chunk matmul weights in lhsT layout.

    wts[c, 128g + p, m] = W[c][m, 128g + p]   (g = 0..3, window blocks)
    wts[c, 512 + p, m]  = L[c][m, p]          (p < 64, correction block)
    """
    n_chunks = n_samples // CHUNK
    t = np.arange(n_samples)
    lag = 1 + z_l[:, None] + np.arange(6)[None, :]
    assert (lag[:, 0] >= CORR).all(), "delay too small for correction width"
    basis = int(lag.max())
    assert basis <= WIN - CORR, "delay too large for window"
    src = t[:, None] - lag
    i_in_chunk = t % CHUNK
    k_win = WIN + i_in_chunk[:, None] - lag
    wts = np.zeros((n_chunks, 5 * CHUNK, CHUNK), np.float32)
    c_of_t = t // CHUNK
    for j in range(6):
        valid = src[:, j] >= 0
        kw = k_win[:, j]
        in_window = valid & (kw < WIN)
        tw = t[in_window]
        wts[c_of_t[tw], kw[tw], i_in_chunk[tw]] += vals[tw, j]
        in_chunk = valid & (kw >= WIN)
        tc = t[in_chunk]
        kc = kw[tc] - WIN
        assert (kc < CORR).all()
        wts[c_of_t[tc], WIN + kc, i_in_chunk[tc]] += vals[tc, j]
    return wts, basis


def _fold_corr(wts_seg):
    """Fold each chunk's within-chunk correction into the weights of its
    in-segment readers so the ring stores *uncorrected* columns."""
    wts_seg = wts_seg.copy()
    n = wts_seg.shape[0]
    blocks = wts_seg.reshape(n, 5, CHUNK, CHUNK)
    corr_active = np.abs(blocks[:, 4]).reshape(n, -1).max(-1) > 0
    for w in range(n):
        if not corr_active[w]:
            continue
        corrT = blocks[w, 4]
        for r in range(w + 1, min(w + 5, n)):
            g = w - r + 4
            blk = blocks[r, g]
            blk[0:CORR] -= corrT[0:CORR, CORR:] @ blk[CORR:]
    return wts_seg


def _basis_ring0(basis, nr):
    """Initial window columns: basis b is a unit at window position
    (WIN-basis)+b; particular and pad columns start at zero."""
    r0 = np.zeros((CHUNK, 4, nr), np.float32)
    for b in range(basis):
        p = (WIN - basis) + b
        r0[p % CHUNK, p // CHUNK, b] = 1.0
    return r0


# ----------------------------------------------------------------------------
# plan construction (shared across cores; SPMD program)
# ----------------------------------------------------------------------------

def _assign_segments(seg_wts_neg):
    """Assign the 32 segments to the (core, chain) grid so that each chain
    slot's 8 segments have similar delay (tighter SPMD union plans).
    Returns assign[s, q] = segment id."""
    act = np.stack([
        np.abs(w.reshape(CPC, 5, -1)).max(-1) > 0 for w in seg_wts_neg
    ])[:, :, :4]  # [SEGS, CPC, 4]
    # initial groups: sort by each segment's dominant window-block pattern
    score = (act * np.arange(1, 5)[None, None, :]).sum((1, 2))
    order = np.argsort(score, kind="stable")
    groups = [order[8 * q: 8 * q + 8].tolist() for q in range(CH)]

    def gcost(g):
        return act[list(g)].any(0).sum()

    costs = [gcost(g) for g in groups]
    import random
    rng = random.Random(0)
    for _ in range(2500):
        a, b = rng.randrange(CH), rng.randrange(CH)
        if a == b:
            continue
        i, j = rng.randrange(N_CORES), rng.randrange(N_CORES)
        groups[a][i], groups[b][j] = groups[b][j], groups[a][i]
        ca, cb = gcost(groups[a]), gcost(groups[b])
        if ca + cb <= costs[a] + costs[b]:
            costs[a], costs[b] = ca, cb
        else:
            groups[a][i], groups[b][j] = groups[b][j], groups[a][i]
    # segment 0 carries the excitation; the program injects x only at
    # chain-0 slots, so its group must be chain 0
    q0 = next(q for q in range(CH) if 0 in groups[q])
    groups[0], groups[q0] = groups[q0], groups[0]
    assign = np.zeros((N_CORES, CH), np.int64)
    for q in range(CH):
        for s in range(N_CORES):
            assign[s, q] = groups[q][s]
    return assign


def _make_plans(seg_wts_neg, assign):
    """Per-slot (q, r) union plans across cores.

    Returns:
      plans[q][r] = (wblocks tuple, corr_t bool)   # phase B
      corr_y[q][r] = bool                          # apply fixup positions
    """
    act = np.stack([
        np.abs(w.reshape(CPC, 5, -1)).max(-1) > 0 for w in seg_wts_neg
    ])  # [SEGS, CPC, 5]
    plans = []
    corr_y = []
    for q in range(CH):
        segs = [assign[s, q] for s in range(N_CORES)]
        u = act[segs].any(0)  # [CPC, 5]
        pq = []
        cq = []
        for r in range(CPC):
            wb = tuple(g for g in range(4) if u[r, g])
            if not wb:
                wb = (3,)
            pq.append((wb, False))
            cq.append(bool(u[r, 4]))
        plans.append(pq)
        corr_y.append(cq)
    return plans, corr_y


def _pack_weights(seg_wts_neg, plans, assign, s):
    """Pack core s's phase-B weight blocks, round-major, partition-major
    fp16 layout [128, TOT, 128]."""
    cols = []
    for r in range(CPC):
        for q in range(CH):
            wb, corr_t = plans[q][r]
            blocks = seg_wts_neg[assign[s, q]].reshape(CPC, 5, CHUNK, CHUNK)
            sel = list(wb) + ([4] if corr_t else [])
            cols.append(blocks[r, sel])  # [nb, 128, 128]
    flat = np.concatenate(cols, 0)       # [TOT, 128k, 128m]
    return np.ascontiguousarray(
        flat.transpose(1, 0, 2)).astype(np.float16)  # [128, TOT, 128]


# ----------------------------------------------------------------------------
# phase B program
# ----------------------------------------------------------------------------

def _plan_key(plans):
    return tuple(tuple((wb, co) for wb, co in pq) for pq in plans)


def _build_phaseb_nc(plans, tot_blocks, basis):
    upd = os.environ.get("DIFFKS_UPD", "act")   # act | stt | vv
    hq = os.environ.get("DIFFKS_HQ", "sync")  # gpsimd | sync
    us_env = os.environ.get("DIFFKS_US1", "216")
    key = ("B2", _plan_key(plans), tot_blocks, basis, upd, hq, us_env)
    if key in _NC_CACHE:
        return _NC_CACHE[key]
    nr, pidx = _nr_of(basis)

    nb_round_max = max(
        sum(len(plans[q][r][0]) + int(plans[q][r][1]) for q in range(CH))
        for r in range(CPC)
    )
    nc = bacc.Bacc("TRN2", target_bir_lowering=False, debug=False,
                   num_devices=N_CORES, enable_partition_id=False)
    wts = nc.dram_tensor("wts", [CHUNK, tot_blocks, CHUNK], F16,
                         kind="ExternalInput")
    xin = nc.dram_tensor("xin", [CHUNK, CH * CPC], F32, kind="ExternalInput")
    ring0 = nc.dram_tensor("ring0", [CHUNK, 4, nr], F16,
                           kind="ExternalInput")
    hout = nc.dram_tensor("hout", [CHUNK, CH * CPC, nr], F16,
                          kind="ExternalOutput")

    with tile.TileContext(nc) as tc:
        with (
            tc.tile_pool(name="state", bufs=1) as state,
            tc.tile_pool(name="wpool", bufs=4) as wpool,
            tc.tile_pool(name="psum", bufs=8, space="PSUM") as ppool,
        ):
            ring = state.tile([CHUNK, CH, RING, nr], F16)
            xin_sb = state.tile([CHUNK, CH * CPC], F32)
            xext = None
            if upd in ("stt", "vv"):
                xext = state.tile([CHUNK, CH, nr], F16)
                nc.vector.memset(xext[:], 0.0)
            woff = 0
            wtile0 = None
            for r in range(CPC):
                nbr = sum(len(plans[q][r][0]) + int(plans[q][r][1])
                          for q in range(CH))
                wtile = wpool.tile([CHUNK, nb_round_max, CHUNK], F16, tag="w")
                if r == 0:
                    # round 0: per-slot weight pieces interleaved with the
                    # ring0 columns so the first matmul starts ~2us earlier
                    so = 0
                    for q in range(CH):
                        nb_q = len(plans[q][0][0]) + int(plans[q][0][1])
                        nc.sync.dma_start(
                            wtile[:, so: so + nb_q, :],
                            wts[:, woff + so: woff + so + nb_q, :])
                        nc.sync.dma_start(ring[:, q, 4:8, :], ring0[:])
                        so += nb_q
                    nc.sync.dma_start(xin_sb[:], xin[:])
                else:
                    nc.sync.dma_start(wtile[:, 0:nbr, :],
                                      wts[:, woff: woff + nbr, :])
                woff += nbr
                soff = 0
                for q in range(CH):
                    wb, corr_t = plans[q][r]
                    slot = r * CH + q
                    rc = r % RING
                    psum = ppool.tile([CHUNK, nr], F32, tag="acc")
                    for i, g in enumerate(wb):
                        col = (r + 4 + g) % RING
                        nc.tensor.matmul(
                            psum[:],
                            wtile[:, soff + i, :],
                            ring[:, q, col, :],
                            start=(i == 0),
                            stop=(i == len(wb) - 1),
                        )
                    # serial ring update (weights pre-negated: col = psum + x).
                    # One balanced copy each on ACT and Vector; x is nonzero
                    # only for the first 4 chunks of segment 0, so only those
                    # slots get an in-place x-add (other cores add zero).
                    if upd == "act":
                        s1 = int(os.environ.get("DIFFKS_US1", "216"))
                        nc.scalar.copy(ring[:, q, rc, 0:s1], psum[:, 0:s1])
                        nc.vector.tensor_copy(ring[:, q, rc, s1:nr],
                                              psum[:, s1:nr])
                        if q == 0 and r < 4:
                            nc.vector.tensor_add(
                                ring[:, q, rc, pidx: pidx + 1],
                                ring[:, q, rc, pidx: pidx + 1],
                                xin_sb[:, slot: slot + 1],
                            )
                    else:
                        if upd == "stt":
                            nc.scalar.copy(xext[:, q, pidx: pidx + 1],
                                           xin_sb[:, slot: slot + 1])
                        else:
                            nc.vector.tensor_copy(xext[:, q, pidx: pidx + 1],
                                                  xin_sb[:, slot: slot + 1])
                        nc.vector.scalar_tensor_tensor(
                            out=ring[:, q, rc, :], in0=psum[:], scalar=1.0,
                            in1=xext[:, q, :], op0=mybir.AluOpType.mult,
                            op1=mybir.AluOpType.add,
                        )
                    soff += len(wb) + int(corr_t)
                    if r % 4 == 3:
                        base = (r - 3) % RING
                        dmaeng = nc.gpsimd if hq == "gpsimd" else nc.sync
                        dmaeng.dma_start(
                            hout[:, q * CPC + (r - 3): q * CPC + r + 1, :],
                            ring[:, q, base: base + 4, :],
                        )
    nc.compile()
    _NC_CACHE[key] = nc
    return nc


# ----------------------------------------------------------------------------
# apply program
# ----------------------------------------------------------------------------

def _build_apply_nc(corr_slots, nrhs):
    acc = os.environ.get("DIFFKS_ACC", "vec")   # tri | vec
    ncorrmode = os.environ.get("DIFFKS_NCORR", "on")  # on | off
    key = ("A2", tuple(corr_slots), nrhs, acc, ncorrmode)
    if key in _NC_CACHE:
        return _NC_CACHE[key]
    n_corr = max(len(corr_slots), 1)
    nc = bacc.Bacc("TRN2", target_bir_lowering=False, debug=False,
                   num_devices=N_CORES, enable_partition_id=False)
    hseg = nc.dram_tensor("hseg", [CHUNK, CH * CPC, nrhs], F16,
                          kind="ExternalInput")
    wb = nc.dram_tensor("wb", [CHUNK, CH, nrhs], F16, kind="ExternalInput")
    yout = nc.dram_tensor("yout", [CHUNK, CH * CPC], F32,
                          kind="ExternalOutput")

    with tile.TileContext(nc) as tc:
        with (
            tc.tile_pool(name="state", bufs=1) as state,
            tc.tile_pool(name="hpool", bufs=6) as hpool,
            tc.tile_pool(name="spool", bufs=4) as spool,
        ):
            wb_sb = state.tile([CHUNK, CH, nrhs], F16)
            nc.sync.dma_start(wb_sb[:], wb[:])
            yout_sb = state.tile([CHUNK, CH * CPC], F32)
            # fused multiply + accumulate-reduce per H column (corr fix-ups
            # are applied on the host after yout returns)
            for grp in range(CH * CPC // 4):
                htile = hpool.tile([CHUNK, 4, nrhs], F16, tag="h")
                nc.sync.dma_start(htile[:],
                                  hseg[:, grp * 4:(grp + 1) * 4, :])
                for j in range(4):
                    hcol = grp * 4 + j
                    q = hcol // CPC
                    scratch = spool.tile([CHUNK, nrhs], F16, tag="s")
                    if acc == "tri" and hcol % 3 == 0:
                        # gpsimd computes the products, ACT reduce-accumulates
                        nc.gpsimd.scalar_tensor_tensor(
                            out=scratch[:], in0=htile[:, j, :], scalar=1.0,
                            in1=wb_sb[:, q, :], op0=mybir.AluOpType.mult,
                            op1=mybir.AluOpType.mult,
                        )
                        scr2 = spool.tile([CHUNK, nrhs], F16, tag="s2")
                        nc.scalar.activation(
                            out=scr2[:], in_=scratch[:],
                            func=mybir.ActivationFunctionType.Identity,
                            accum_out=yout_sb[:, hcol: hcol + 1],
                        )
                    else:
                        nc.vector.scalar_tensor_tensor(
                            out=scratch[:], in0=htile[:, j, :], scalar=1.0,
                            in1=wb_sb[:, q, :], op0=mybir.AluOpType.mult,
                            op1=mybir.AluOpType.mult,
                            accum_out=yout_sb[:, hcol: hcol + 1],
                        )
            nc.sync.dma_start(yout[:], yout_sb[:])
    nc.compile()
    _NC_CACHE[key] = nc
    return nc


# ----------------------------------------------------------------------------
# host orchestration
# ----------------------------------------------------------------------------

def _run(nc, in_maps, tag):
    trace = bool(int(os.environ.get("DIFFKS_TRACE", "0")))
    kw = {}
    tcs = os.environ.get("DIFFKS_TRACE_CORES", "")
    if trace and tcs:
        kw["trace_cores"] = [int(x) for x in tcs.split(",")]
    res = run_bass_kernel_spmd(
        nc, in_maps, core_ids=list(range(len(in_maps))), trace=trace, **kw
    )
    LAST_RESULTS[tag] = res
    return res.results


def kernel(delay_len_frames, raw_coeff_frames, excitation, n_samples):
    n = int(n_samples)
    assert n == N_SAMPLES, f"kernel hardcoded for {N_SAMPLES}, got {n}"
    LAST_RESULTS.clear()

    vals, z_l, x = _preprocess(delay_len_frames, raw_coeff_frames,
                               excitation, n)
    wts, basis = _build_wts(vals, z_l, n)
    nr, pidx = _nr_of(basis)
    n_chunks = n // CHUNK
    assert n_chunks == SEGS * CPC
    xin_cols = np.ascontiguousarray(x.reshape(n_chunks, CHUNK).T)  # [128, nc]

    # fold corrections, then negate everything (update becomes plain copy)
    seg_wts_neg = [-_fold_corr(wts[j * CPC:(j + 1) * CPC])
                   for j in range(SEGS)]
    assign = _assign_segments(seg_wts_neg)
    inv = {int(assign[s, q]): (s, q)
           for s in range(N_CORES) for q in range(CH)}
    plans, corr_y = _make_plans(seg_wts_neg, assign)
    tot_blocks = sum(len(plans[q][r][0]) + int(plans[q][r][1])
                     for r in range(CPC) for q in range(CH))

    ncB = _build_phaseb_nc(plans, tot_blocks, basis)
    r0 = _basis_ring0(basis, nr).astype(np.float16)
    in_maps = []
    for s in range(N_CORES):
        xin = np.zeros((CHUNK, CH * CPC), np.float32)
        for r in range(CPC):
            for q in range(CH):
                gchunk = int(assign[s, q]) * CPC + r
                xin[:, r * CH + q] = xin_cols[:, gchunk]
        in_maps.append({
            "wts": _pack_weights(seg_wts_neg, plans, assign, s),
            "xin": xin,
            "ring0": r0,
        })
    outsB = _run(ncB, in_maps, "phaseB")

    # host combine: build each segment's transfer operator from its last 4
    # (uncorrected) H columns + the correction blocks, then chain them (fp32)
    wins = [np.zeros(WIN, np.float32)]
    for j in range(SEGS):
        s, q = inv[j]
        base = q * CPC + (CPC - 4)
        T = outsB[s]["hout"][:, base: base + 4, :].astype(np.float32)
        blocks = seg_wts_neg[j].reshape(CPC, 5, CHUNK, CHUNK)
        for k in range(4):
            Lc = blocks[CPC - 4 + k, 4][0:CORR]      # negated lhsT [64, 128]
            if np.any(Lc):
                fix = Lc.T @ T[0:CORR, k, :]          # [128, nr]
                T[CORR:, k, :] += fix[CORR:]
        T = T.transpose(1, 0, 2).reshape(WIN, nr)
        w_next = T[:, :basis] @ wins[j][WIN - basis:] + T[:, pidx]
        wins.append(w_next.astype(np.float32))

    # apply: y[:, c] = H_c @ [w; 1]
    corr_slots = [q * CPC + r for q in range(CH) for r in range(CPC)
                  if corr_y[q][r]]
    ncA = _build_apply_nc(corr_slots, nr)
    n_corr = max(len(corr_slots), 1)
    in_maps = []
    for s in range(N_CORES):
        wbv = np.zeros((CHUNK, CH, nr), np.float16)
        for q in range(CH):
            j = int(assign[s, q])
            wv = np.zeros(nr, np.float32)
            wv[:basis] = wins[j][WIN - basis:]
            wv[pidx] = 1.0
            wbv[:, q, :] = wv.astype(np.float16)[None, :]
        in_maps.append({
            "hseg": outsB[s]["hout"],
            "wb": wbv,
        })
    outsA = _run(ncA, in_maps, "apply")

    y = np.zeros(n, np.float32)
    for s in range(N_CORES):
        yo = np.array(outsA[s]["yout"])          # [128, CH*CPC]
        for q in range(CH):
            blocks = seg_wts_neg[int(assign[s, q])].reshape(
                CPC, 5, CHUNK, CHUNK)
            for r in range(CPC):
                hcol = q * CPC + r
                Lc = blocks[r, 4][0:CORR]        # negated lhsT [64, 128]
                if np.any(Lc):
                    fix = Lc.T @ yo[0:CORR, hcol]
                    yo[CORR:, hcol] += fix[CORR:]
                gchunk = int(assign[s, q]) * CPC + r
                y[gchunk * CHUNK:(gchunk + 1) * CHUNK] = yo[:, hcol]
    return y.astype(np.float32)


# revision 27
# speedup vs baseline: 1.0514x; 1.0514x over previous
"""Trainium2 Bass kernel for nn_DiffKS (differentiable Karplus-Strong).

Structure of the computation:
  y[t] = x[t] - sum_{j=0..5} vals[t,j] * y[t - 1 - z_l[t] - j]
with vals / z_l derived from spline-interpolated delay & coefficient
trajectories.  The feedback lag (1 + z_l + j) is always >= ~93 samples, so
128-sample chunks can be computed as dense banded matmuls against a
512-sample window of past output plus a small within-chunk correction.

Parallel structure (v2):
  - 32 time segments of 2048 samples; each of the 8 cores runs 4 segments
    as INDEPENDENT interleaved chunk-chains (4 chains x 16 rounds), which
    keeps the tensor engine busy while each chain's serial
    matmul->ring-update link completes.  All tensors fp16 (weights, ring,
    H) with fp32 PSUM accumulation: host-simulated rel err ~5e-4.
  - phase B (parallel): each chain runs its segment's chunked recurrence
    with basis+1 right-hand sides (unit initial-window columns + one
    particular column).  Ring columns double as the response operator H
    (streamed to DRAM in fp16); the corrected final windows form the
    segment's transfer operator T.
  - combine (host, tiny): chain the 32 transfer operators to get every
    segment's true initial window.
  - apply (parallel): y[:, c] = H_c @ [w; 1] as fused multiply+
    accumulate-reduce ops split across the Vector and GpSimd engines,
    plus within-chunk correction fix-ups.

Weights are pre-negated on the host so the serial ring update is a plain
PSUM->SBUF copy on the Scalar (ACT) engine (plus a [128,1] x-column add on
Vector), keeping the Vector engine off the critical chain.
"""

import os
import numpy as np

import concourse.bacc as bacc
import concourse.tile as tile
import concourse.mybir as mybir
from concourse.bass_utils import run_bass_kernel_spmd


def _ensure_ntff_hook():
    """The agent image's `antenv` stub lacks `axon_hooks`, which
    `run_bass_kernel_spmd(trace=True)` needs under axon for NTFF capture."""
    try:
        from antenv.axon_hooks import get_axon_ntff_profile_hook  # noqa: F401
        return
    except ImportError:
        pass
    import contextlib
    import ctypes
    import sys
    import types

    so_path = "/opt/axon/libaxon_pjrt.so"
    if not os.path.exists(so_path):
        return
    lib = ctypes.CDLL(so_path)
    if not hasattr(lib, "axon_start_nrt_profile"):
        return
    lib.axon_start_nrt_profile.argtypes = [
        ctypes.POINTER(ctypes.c_int64), ctypes.c_size_t]
    lib.axon_start_nrt_profile.restype = ctypes.c_int64
    lib.axon_stop_nrt_profile.argtypes = [ctypes.c_char_p]
    lib.axon_stop_nrt_profile.restype = ctypes.c_int64

    @contextlib.contextmanager
    def _hook(output_dir, device_ids):
        import jax
        jax.devices()
        if device_ids:
            ids = (ctypes.c_int64 * len(device_ids))(*device_ids)
            rc = lib.axon_start_nrt_profile(ids, len(device_ids))
        else:
            rc = lib.axon_start_nrt_profile(None, 0)
        if rc != 0:
            raise RuntimeError(f"axon_start_nrt_profile rc={rc}")
        try:
            yield
        finally:
            n = lib.axon_stop_nrt_profile(str(output_dir).encode())
            if n <= 0:
                print(f"ntff profile: {n} file(s) written to {output_dir}",
                      file=sys.stderr)

    mod = types.ModuleType("antenv.axon_hooks")
    mod._hook = _hook
    mod.get_axon_ntff_profile_hook = lambda: _hook
    mod.set_axon_ntff_profile_hook = lambda h: setattr(mod, "_hook", h)
    import antenv
    antenv.axon_hooks = mod
    sys.modules["antenv.axon_hooks"] = mod


_ensure_ntff_hook()

F32 = mybir.dt.float32
F16 = mybir.dt.float16

N_SAMPLES = 65536
N_FRAMES = 64
L_ORDER = 5
CHUNK = 128
WIN = 512            # window length the chunk matmuls see (4 ring cols)
RING = 8             # ring columns per chain in SBUF
CORR = 64            # within-chunk correction width (needs z_l >= 63)
N_CORES = 8
CH = 4               # independent chains (segments) per core
CPC = 16             # chunks (rounds) per chain
SEGS = N_CORES * CH  # 32 segments of 2048 samples

# filled by kernel() with per-phase profiling results for the test harness
LAST_RESULTS = {}

_NC_CACHE = {}

# device rhs layout: basis columns [0, basis); one zero pad column; the
# particular column at PIDX (4-byte aligned for the fp16 column update);
# one trailing pad so the total width is even.
def _nr_of(basis):
    pidx = basis + 1 + (basis + 1) % 2
    return pidx + 2 - (basis + 1) % 2, pidx


def _seg_of(s, q):
    """Segment index handled by core s, chain q."""
    return s + N_CORES * q


# ----------------------------------------------------------------------------
# host-side preprocessing
# ----------------------------------------------------------------------------

_SPLINE_CACHE = {}


def _spline_matrix(n_in, n_out):
    """Static [n_out, n_in] natural-cubic-spline interpolation matrix."""
    key = (n_in, n_out)
    if key in _SPLINE_CACHE:
        return _SPLINE_CACHE[key]
    t_in = np.linspace(0.0, 1.0, n_in)
    t_out = np.linspace(0.0, 1.0, n_out)
    n = n_in
    h = t_in[1:] - t_in[:-1]
    R = np.zeros((n - 2, n))
    for i in range(n - 2):
        R[i, i] += 6.0 / h[i]
        R[i, i + 1] += -6.0 / h[i] - 6.0 / h[i + 1]
        R[i, i + 2] += 6.0 / h[i + 1]
    A = (
        np.diag(2.0 * (h[:-1] + h[1:]))
        + np.diag(h[1:-1], 1)
        + np.diag(h[1:-1], -1)
    )
    M = np.zeros((n, n))
    M[1:-1] = np.linalg.solve(A, R)
    idx = np.clip(np.searchsorted(t_in, t_out, side="right") - 1, 0, n - 2)
    dt = t_out - t_in[idx]
    S = np.zeros((n_out, n))
    eye = np.eye(n)
    for r in range(n_out):
        i = idx[r]
        b = (eye[i + 1] - eye[i]) / h[i] - h[i] * (2.0 * M[i] + M[i + 1]) / 6.0
        c = M[i] / 2.0
        d = (M[i + 1] - M[i]) / (6.0 * h[i])
        S[r] = eye[i] + b * dt[r] + c * dt[r] ** 2 + d * dt[r] ** 3
    S = S.astype(np.float32)
    _SPLINE_CACHE[key] = S
    return S


def _preprocess(delay, raw, exc, n_samples):
    sig = 1.0 / (1.0 + np.exp(-np.asarray(raw, np.float32)))
    coeff = sig / sig.sum(-1, keepdims=True)
    S = _spline_matrix(N_FRAMES, n_samples)
    delay_interp = S @ np.asarray(delay, np.float32)
    coeff_interp = S @ coeff
    z_l = np.floor(delay_interp).astype(np.int32)
    alfa = (delay_interp - z_l).astype(np.float32)
    b = coeff_interp
    v0 = -(1.0 - alfa) * b[:, 0]
    vmid = -(alfa[:, None] * b[:, : L_ORDER - 1]
             + (1.0 - alfa)[:, None] * b[:, 1:L_ORDER])
    vL = -alfa * b[:, -1]
    vals = np.concatenate([v0[:, None], vmid, vL[:, None]], 1).astype(np.float32)
    x = np.zeros(n_samples, np.float32)
    exc = np.asarray(exc, np.float32)
    x[: exc.shape[0]] = exc
    return vals, z_l, x


def _build_wts(vals, z_l, n_samples):
    """Dense per-chunk matmul weights in lhsT layout.

    wts[c, 128g + p, m] = W[c][m, 128g + p]   (g = 0..3, window blocks)
    wts[c, 512 + p, m]  = L[c][m, p]          (p < 64, correction block)
    """
    n_chunks = n_samples // CHUNK
    t = np.arange(n_samples)
    lag = 1 + z_l[:, None] + np.arange(6)[None, :]
    assert (lag[:, 0] >= CORR).all(), "delay too small for correction width"
    basis = int(lag.max())
    assert basis <= WIN - CORR, "delay too large for window"
    src = t[:, None] - lag
    i_in_chunk = t % CHUNK
    k_win = WIN + i_in_chunk[:, None] - lag
    wts = np.zeros((n_chunks, 5 * CHUNK, CHUNK), np.float32)
    c_of_t = t // CHUNK
    for j in range(6):
        valid = src[:, j] >= 0
        kw = k_win[:, j]
        in_window = valid & (kw < WIN)
        tw = t[in_window]
        wts[c_of_t[tw], kw[tw], i_in_chunk[tw]] += vals[tw, j]
        in_chunk = valid & (kw >= WIN)
        tc = t[in_chunk]
        kc = kw[tc] - WIN
        assert (kc < CORR).all()
        wts[c_of_t[tc], WIN + kc, i_in_chunk[tc]] += vals[tc, j]
    return wts, basis


def _fold_corr(wts_seg):
    """Fold each chunk's within-chunk correction into the weights of its
    in-segment readers so the ring stores *uncorrected* columns."""
    wts_seg = wts_seg.copy()
    n = wts_seg.shape[0]
    blocks = wts_seg.reshape(n, 5, CHUNK, CHUNK)
    corr_active = np.abs(blocks[:, 4]).reshape(n, -1).max(-1) > 0
    for w in range(n):
        if not corr_active[w]:
            continue
        corrT = blocks[w, 4]
        for r in range(w + 1, min(w + 5, n)):
            g = w - r + 4
            blk = blocks[r, g]
            blk[0:CORR] -= corrT[0:CORR, CORR:] @ blk[CORR:]
    return wts_seg


def _basis_ring0(basis, nr):
    """Initial window columns: basis b is a unit at window position
    (WIN-basis)+b; particular and pad columns start at zero."""
    r0 = np.zeros((CHUNK, 4, nr), np.float32)
    for b in range(basis):
        p = (WIN - basis) + b
        r0[p % CHUNK, p // CHUNK, b] = 1.0
    return r0


# ----------------------------------------------------------------------------
# plan construction (shared across cores; SPMD program)
# ----------------------------------------------------------------------------

def _assign_segments(seg_wts_neg):
    """Assign the 32 segments to the (core, chain) grid so that each chain
    slot's 8 segments have similar delay (tighter SPMD union plans).
    Returns assign[s, q] = segment id."""
    act = np.stack([
        np.abs(w.reshape(CPC, 5, -1)).max(-1) > 0 for w in seg_wts_neg
    ])[:, :, :4]  # [SEGS, CPC, 4]
    # initial groups: sort by each segment's dominant window-block pattern
    score = (act * np.arange(1, 5)[None, None, :]).sum((1, 2))
    order = np.argsort(score, kind="stable")
    groups = [order[8 * q: 8 * q + 8].tolist() for q in range(CH)]

    def gcost(g):
        return act[list(g)].any(0).sum()

    costs = [gcost(g) for g in groups]
    import random
    rng = random.Random(0)
    for _ in range(2500):
        a, b = rng.randrange(CH), rng.randrange(CH)
        if a == b:
            continue
        i, j = rng.randrange(N_CORES), rng.randrange(N_CORES)
        groups[a][i], groups[b][j] = groups[b][j], groups[a][i]
        ca, cb = gcost(groups[a]), gcost(groups[b])
        if ca + cb <= costs[a] + costs[b]:
            costs[a], costs[b] = ca, cb
        else:
            groups[a][i], groups[b][j] = groups[b][j], groups[a][i]
    # segment 0 carries the excitation; the program injects x only at
    # chain-0 slots, so its group must be chain 0
    q0 = next(q for q in range(CH) if 0 in groups[q])
    groups[0], groups[q0] = groups[q0], groups[0]
    assign = np.zeros((N_CORES, CH), np.int64)
    for q in range(CH):
        for s in range(N_CORES):
            assign[s, q] = groups[q][s]
    return assign


def _make_plans(seg_wts_neg, assign):
    """Per-slot (q, r) union plans across cores.

    Returns:
      plans[q][r] = (wblocks tuple, corr_t bool)   # phase B
      corr_y[q][r] = bool                          # apply fixup positions
    """
    act = np.stack([
        np.abs(w.reshape(CPC, 5, -1)).max(-1) > 0 for w in seg_wts_neg
    ])  # [SEGS, CPC, 5]
    plans = []
    corr_y = []
    for q in range(CH):
        segs = [assign[s, q] for s in range(N_CORES)]
        u = act[segs].any(0)  # [CPC, 5]
        pq = []
        cq = []
        for r in range(CPC):
            wb = tuple(g for g in range(4) if u[r, g])
            if not wb:
                wb = (3,)
            pq.append((wb, False))
            cq.append(bool(u[r, 4]))
        plans.append(pq)
        corr_y.append(cq)
    return plans, corr_y


def _pack_weights(seg_wts_neg, plans, assign, s):
    """Pack core s's phase-B weight blocks, round-major, partition-major
    fp16 layout [128, TOT, 128]."""
    cols = []
    for r in range(CPC):
        for q in range(CH):
            wb, corr_t = plans[q][r]
            blocks = seg_wts_neg[assign[s, q]].reshape(CPC, 5, CHUNK, CHUNK)
            sel = list(wb) + ([4] if corr_t else [])
            cols.append(blocks[r, sel])  # [nb, 128, 128]
    flat = np.concatenate(cols, 0)       # [TOT, 128k, 128m]
    return np.ascontiguousarray(
        flat.transpose(1, 0, 2)).astype(np.float16)  # [128, TOT, 128]


# ----------------------------------------------------------------------------
# phase B program
# ----------------------------------------------------------------------------

def _plan_key(plans):
    return tuple(tuple((wb, co) for wb, co in pq) for pq in plans)


def _build_phaseb_nc(plans, tot_blocks, basis):
    upd = os.environ.get("DIFFKS_UPD", "act")   # act | stt | vv
    hq = os.environ.get("DIFFKS_HQ", "gpsimd")  # gpsimd | sync
    us_env = os.environ.get("DIFFKS_US1", "216")
    key = ("B2", _plan_key(plans), tot_blocks, basis, upd, hq, us_env)
    if key in _NC_CACHE:
        return _NC_CACHE[key]
    nr, pidx = _nr_of(basis)

    nb_round_max = max(
        sum(len(plans[q][r][0]) + int(plans[q][r][1]) for q in range(CH))
        for r in range(CPC)
    )
    nc = bacc.Bacc("TRN2", target_bir_lowering=False, debug=False,
                   num_devices=N_CORES, enable_partition_id=False)
    wts = nc.dram_tensor("wts", [CHUNK, tot_blocks, CHUNK], F16,
                         kind="ExternalInput")
    xin = nc.dram_tensor("xin", [CHUNK, CH * CPC], F32, kind="ExternalInput")
    ring0 = nc.dram_tensor("ring0", [CHUNK, 4, nr], F16,
                           kind="ExternalInput")
    hout = nc.dram_tensor("hout", [CHUNK, CH * CPC, nr], F16,
                          kind="ExternalOutput")

    with tile.TileContext(nc) as tc:
        with (
            tc.tile_pool(name="state", bufs=1) as state,
            tc.tile_pool(name="wpool", bufs=4) as wpool,
            tc.tile_pool(name="psum", bufs=8, space="PSUM") as ppool,
        ):
            ring = state.tile([CHUNK, CH, RING, nr], F16)
            xin_sb = state.tile([CHUNK, CH * CPC], F32)
            xext = None
            if upd in ("stt", "vv"):
                xext = state.tile([CHUNK, CH, nr], F16)
                nc.vector.memset(xext[:], 0.0)
            woff = 0
            wtile0 = None
            for r in range(CPC):
                nbr = sum(len(plans[q][r][0]) + int(plans[q][r][1])
                          for q in range(CH))
                wtile = wpool.tile([CHUNK, nb_round_max, CHUNK], F16, tag="w")
                if r == 0:
                    # round 0: per-slot weight pieces interleaved with the
                    # ring0 columns so the first matmul starts ~2us earlier
                    so = 0
                    for q in range(CH):
                        nb_q = len(plans[q][0][0]) + int(plans[q][0][1])
                        nc.sync.dma_start(
                            wtile[:, so: so + nb_q, :],
                            wts[:, woff + so: woff + so + nb_q, :])
                        nc.sync.dma_start(ring[:, q, 4:8, :], ring0[:])
                        so += nb_q
                    nc.sync.dma_start(xin_sb[:], xin[:])
                else:
                    nc.sync.dma_start(wtile[:, 0:nbr, :],
                                      wts[:, woff: woff + nbr, :])
                woff += nbr
                soff = 0
                for q in range(CH):
                    wb, corr_t = plans[q][r]
                    slot = r * CH + q
                    rc = r % RING
                    psum = ppool.tile([CHUNK, nr], F32, tag="acc")
                    for i, g in enumerate(wb):
                        col = (r + 4 + g) % RING
                        nc.tensor.matmul(
                            psum[:],
                            wtile[:, soff + i, :],
                            ring[:, q, col, :],
                            start=(i == 0),
                            stop=(i == len(wb) - 1),
                        )
                    # serial ring update (weights pre-negated: col = psum + x).
                    # One balanced copy each on ACT and Vector; x is nonzero
                    # only for the first 4 chunks of segment 0, so only those
                    # slots get an in-place x-add (other cores add zero).
                    if upd == "act":
                        s1 = int(os.environ.get("DIFFKS_US1", "216"))
                        nc.scalar.copy(ring[:, q, rc, 0:s1], psum[:, 0:s1])
                        nc.vector.tensor_copy(ring[:, q, rc, s1:nr],
                                              psum[:, s1:nr])
                        if q == 0 and r < 4:
                            nc.vector.tensor_add(
                                ring[:, q, rc, pidx: pidx + 1],
                                ring[:, q, rc, pidx: pidx + 1],
                                xin_sb[:, slot: slot + 1],
                            )
                    else:
                        if upd == "stt":
                            nc.scalar.copy(xext[:, q, pidx: pidx + 1],
                                           xin_sb[:, slot: slot + 1])
                        else:
                            nc.vector.tensor_copy(xext[:, q, pidx: pidx + 1],
                                                  xin_sb[:, slot: slot + 1])
                        nc.vector.scalar_tensor_tensor(
                            out=ring[:, q, rc, :], in0=psum[:], scalar=1.0,
                            in1=xext[:, q, :], op0=mybir.AluOpType.mult,
                            op1=mybir.AluOpType.add,
                        )
                    soff += len(wb) + int(corr_t)
                    if r % 4 == 3:
                        base = (r - 3) % RING
                        dmaeng = nc.gpsimd if hq == "gpsimd" else nc.sync
                        dmaeng.dma_start(
                            hout[:, q * CPC + (r - 3): q * CPC + r + 1, :],
                            ring[:, q, base: base + 4, :],
                        )
    nc.compile()
    _NC_CACHE[key] = nc
    return nc


# ----------------------------------------------------------------------------
# apply program
# ----------------------------------------------------------------------------

def _build_apply_nc(corr_slots, nrhs):
    acc = os.environ.get("DIFFKS_ACC", "vec")   # vec | va
    ncorrmode = os.environ.get("DIFFKS_NCORR", "on")  # on | off
    key = ("A2", tuple(corr_slots), nrhs, acc, ncorrmode)
    if key in _NC_CACHE:
        return _NC_CACHE[key]
    n_corr = max(len(corr_slots), 1)
    nc = bacc.Bacc("TRN2", target_bir_lowering=False, debug=False,
                   num_devices=N_CORES, enable_partition_id=False)
    hseg = nc.dram_tensor("hseg", [CHUNK, CH * CPC, nrhs], F16,
                          kind="ExternalInput")
    wb = nc.dram_tensor("wb", [CHUNK, CH, nrhs], F16, kind="ExternalInput")
    yout = nc.dram_tensor("yout", [CHUNK, CH * CPC], F32,
                          kind="ExternalOutput")

    with tile.TileContext(nc) as tc:
        with (
            tc.tile_pool(name="state", bufs=1) as state,
            tc.tile_pool(name="hpool", bufs=6) as hpool,
            tc.tile_pool(name="spool", bufs=4) as spool,
        ):
            wb_sb = state.tile([CHUNK, CH, nrhs], F16)
            nc.sync.dma_start(wb_sb[:], wb[:])
            yout_sb = state.tile([CHUNK, CH * CPC], F32)
            # fused multiply + accumulate-reduce per H column (corr fix-ups
            # are applied on the host after yout returns)
            GRPW = 8
            for grp in range(CH * CPC // GRPW):
                htile = hpool.tile([CHUNK, GRPW, nrhs], F16, tag="h")
                nc.sync.dma_start(htile[:],
                                  hseg[:, grp * GRPW:(grp + 1) * GRPW, :])
                for j in range(GRPW):
                    hcol = grp * GRPW + j
                    q = hcol // CPC
                    scratch = spool.tile([CHUNK, nrhs], F16, tag="s")
                    if acc == "va" and hcol % 2 == 0:
                        # vector computes products, ACT reduce-accumulates
                        nc.vector.scalar_tensor_tensor(
                            out=scratch[:], in0=htile[:, j, :], scalar=1.0,
                            in1=wb_sb[:, q, :], op0=mybir.AluOpType.mult,
                            op1=mybir.AluOpType.mult,
                        )
                        scr2 = spool.tile([CHUNK, nrhs], F16, tag="s2")
                        nc.scalar.activation(
                            out=scr2[:], in_=scratch[:],
                            func=mybir.ActivationFunctionType.Identity,
                            accum_out=yout_sb[:, hcol: hcol + 1],
                        )
                    else:
                        nc.vector.scalar_tensor_tensor(
                            out=scratch[:], in0=htile[:, j, :], scalar=1.0,
                            in1=wb_sb[:, q, :], op0=mybir.AluOpType.mult,
                            op1=mybir.AluOpType.mult,
                            accum_out=yout_sb[:, hcol: hcol + 1],
                        )
            nc.sync.dma_start(yout[:], yout_sb[:])
    nc.compile()
    _NC_CACHE[key] = nc
    return nc


# ----------------------------------------------------------------------------
# host orchestration
# ----------------------------------------------------------------------------

def _run(nc, in_maps, tag):
    trace = bool(int(os.environ.get("DIFFKS_TRACE", "0")))
    kw = {}
    tcs = os.environ.get("DIFFKS_TRACE_CORES", "")
    if trace and tcs:
        kw["trace_cores"] = [int(x) for x in tcs.split(",")]
    res = run_bass_kernel_spmd(
        nc, in_maps, core_ids=list(range(len(in_maps))), trace=trace, **kw
    )
    LAST_RESULTS[tag] = res
    return res.results


def kernel(delay_len_frames, raw_coeff_frames, excitation, n_samples):
    n = int(n_samples)
    assert n == N_SAMPLES, f"kernel hardcoded for {N_SAMPLES}, got {n}"
    LAST_RESULTS.clear()

    vals, z_l, x = _preprocess(delay_len_frames, raw_coeff_frames,
                               excitation, n)
    wts, basis = _build_wts(vals, z_l, n)
    nr, pidx = _nr_of(basis)
    n_chunks = n // CHUNK
    assert n_chunks == SEGS * CPC
    xin_cols = np.ascontiguousarray(x.reshape(n_chunks, CHUNK).T)  # [128, nc]

    # fold corrections, then negate everything (update becomes plain copy)
    seg_wts_neg = [-_fold_corr(wts[j * CPC:(j + 1) * CPC])
                   for j in range(SEGS)]
    assign = _assign_segments(seg_wts_neg)
    inv = {int(assign[s, q]): (s, q)
           for s in range(N_CORES) for q in range(CH)}
    plans, corr_y = _make_plans(seg_wts_neg, assign)
    tot_blocks = sum(len(plans[q][r][0]) + int(plans[q][r][1])
                     for r in range(CPC) for q in range(CH))

    ncB = _build_phaseb_nc(plans, tot_blocks, basis)
    r0 = _basis_ring0(basis, nr).astype(np.float16)
    in_maps = []
    for s in range(N_CORES):
        xin = np.zeros((CHUNK, CH * CPC), np.float32)
        for r in range(CPC):
            for q in range(CH):
                gchunk = int(assign[s, q]) * CPC + r
                xin[:, r * CH + q] = xin_cols[:, gchunk]
        in_maps.append({
            "wts": _pack_weights(seg_wts_neg, plans, assign, s),
            "xin": xin,
            "ring0": r0,
        })
    outsB = _run(ncB, in_maps, "phaseB")

    # host combine: build each segment's transfer operator from its last 4
    # (uncorrected) H columns + the correction blocks, then chain them (fp32)
    wins = [np.zeros(WIN, np.float32)]
    for j in range(SEGS):
        s, q = inv[j]
        base = q * CPC + (CPC - 4)
        T = outsB[s]["hout"][:, base: base + 4, :].astype(np.float32)
        blocks = seg_wts_neg[j].reshape(CPC, 5, CHUNK, CHUNK)
        for k in range(4):
            Lc = blocks[CPC - 4 + k, 4][0:CORR]      # negated lhsT [64, 128]
            if np.any(Lc):
                fix = Lc.T @ T[0:CORR, k, :]          # [128, nr]
                T[CORR:, k, :] += fix[CORR:]
        T = T.transpose(1, 0, 2).reshape(WIN, nr)
        w_next = T[:, :basis] @ wins[j][WIN - basis:] + T[:, pidx]
        wins.append(w_next.astype(np.float32))

    # apply: y[:, c] = H_c @ [w; 1]
    corr_slots = [q * CPC + r for q in range(CH) for r in range(CPC)
                  if corr_y[q][r]]
    ncA = _build_apply_nc(corr_slots, nr)
    n_corr = max(len(corr_slots), 1)
    in_maps = []
    for s in range(N_CORES):
        wbv = np.zeros((CHUNK, CH, nr), np.float16)
        for q in range(CH):
            j = int(assign[s, q])
            wv = np.zeros(nr, np.float32)
            wv[:basis] = wins[j][WIN - basis:]
            wv[pidx] = 1.0
            wbv[:, q, :] = wv.astype(np.float16)[None, :]
        in_maps.append({
            "hseg": outsB[s]["hout"],
            "wb": wbv,
        })
    outsA = _run(ncA, in_maps, "apply")

    y = np.zeros(n, np.float32)
    for s in range(N_CORES):
        yo = np.array(outsA[s]["yout"])          # [128, CH*CPC]
        for q in range(CH):
            blocks = seg_wts_neg[int(assign[s, q])].reshape(
                CPC, 5, CHUNK, CHUNK)
            for r in range(CPC):
                hcol = q * CPC + r
                Lc = blocks[r, 4][0:CORR]        # negated lhsT [64, 128]
                if np.any(Lc):
                    fix = Lc.T @ yo[0:CORR, hcol]
                    yo[CORR:, hcol] += fix[CORR:]
                gchunk = int(assign[s, q]) * CPC + r
                y[gchunk * CHUNK:(gchunk + 1) * CHUNK] = yo[:, hcol]
    return y.astype(np.float32)
